# revision 26
# baseline (speedup 1.0000x reference)
"""BinaryLinear Trainium2 kernel: y = x @ sign(W).T + bias.

Full shapes: x [8192, 2048] f32, W [2048, 2048] f32, bias [2048] f32.
Strategy: data-parallel over 8 NeuronCores — shard x rows (1024/core),
replicate W and bias, no collectives. Host only shards / lays out /
down-casts to the kernel's bf16 compute precision (sign is preserved
exactly by the bf16 cast); all math (sign, matmul, bias add) runs on
device.

Numerics: W is binarized on-device to {-0.5, +0.5} in bf16 via
(w >= 0) - 0.5 (one DVE op, in place); the missing factor 2 is folded
into the fp32 PSUM eviction (out = 2*psum + bias, one DVE op). Both
factors are powers of two, so the result equals x*sign(W) exactly up to
the single bf16 rounding of x. Accumulation is fp32 in PSUM (K=2048).

Schedule: W streams in 512-out-col strips, host-packed as
[strip, partition, k, col] so every DMA line is >=2KB contiguous. Each
strip is computed K-outer across 8 PSUM banks (one per 128-row x
block), so the TensorE consumes chunks as they arrive and never waits
on the full W. Only the first W chunk + two x K-tiles ride the sync DMA
queue (kept shallow — DGE completions retire in order, so a deep ring
delays the critical first tiles); the bulk streams on the scalar
engine's queue in consumption order. Binarize is emitted so strip-n
evictions never queue behind later strips' binarize on the DVE. Warmup
matmuls on a scratch tile lift the PE clock gate before real data
lands. Output DMAs alternate between the sync and scalar HWDGE queues
(SWDGE drain at kernel end is slow).
"""

import numpy as np
import ml_dtypes

N_CORES = 8
N_ROWS = 8192
D_IN = 2048
D_OUT = 2048
N_SH = N_ROWS // N_CORES

KB = 128            # contraction block (SBUF partitions)
MB = 128            # x-row block (stationary free dim -> out partitions)
NB = 512            # out-col block (moving free dim, one PSUM bank)

_cache = {}


def _chunk_sizes(nk, first_strip):
    # strip 0 uses small leading chunks so the first matmul starts early
    sizes = []
    k = 0
    while k < nk:
        if first_strip:
            csz = 1 if len(sizes) < 2 else 2
        else:
            csz = 8
        s = min(csz, nk - k)
        sizes.append(s)
        k += s
    return sizes


def build_nc(nsh=N_SH, din=D_IN, dout=D_OUT, warmup_mms=8):
    import concourse.bass as bass
    import concourse.bacc as bacc
    import concourse.tile as tile
    from concourse import mybir

    f32 = mybir.dt.float32
    bf16 = mybir.dt.bfloat16

    nk = din // KB
    nm = nsh // MB
    nn = dout // NB
    assert nm <= 8, "one PSUM bank per x-row block"

    nc = bacc.Bacc("TRN2", debug=False)
    xt = nc.dram_tensor("xt", [din, nsh], bf16, kind="ExternalInput").ap()
    wt4 = nc.dram_tensor("wt4", [nn, KB, nk, NB], bf16, kind="ExternalInput").ap()
    bias = nc.dram_tensor("bias", [dout], f32, kind="ExternalInput").ap()
    y = nc.dram_tensor("y", [nsh, dout], f32, kind="ExternalOutput").ap()

    with tile.TileContext(nc) as tc:
        with (
            tc.tile_pool(name="wb", bufs=1) as wb_pool,
            tc.tile_pool(name="xb", bufs=1) as xb_pool,
            tc.tile_pool(name="biasp", bufs=1) as bias_pool,
            tc.tile_pool(name="out", bufs=8) as out_pool,
            tc.tile_pool(name="psum", bufs=8, space=bass.MemorySpace.PSUM) as psum_pool,
        ):
            # PE clock-gate warmup on a zeroed scratch tile
            if warmup_mms:
                dummy = bias_pool.tile([128, NB], bf16, tag="dummy")
                nc.vector.memset(dummy[:, :], 0.0)
                wps = psum_pool.tile([128, NB], f32, tag="ps", name="ps_warm")
                for _ in range(warmup_mms):
                    nc.tensor.matmul(
                        wps[:, :], dummy[:, 0:MB], dummy[:, :],
                        start=True, stop=True,
                    )

            # Input DMAs in exact consumption order. Only the first W chunk
            # and its two x K-tiles go on the sync queue (kept shallow so
            # their completion semaphores retire fast); the rest streams on
            # the scalar engine's queue, self-pacing at full bandwidth.
            bias_bc = bias_pool.tile([128, dout], f32, tag="biasbc")
            xb = []
            wb = {}          # (n, k) -> (chunk tile, local k index)
            strip_chunks = [[] for _ in range(nn)]
            for n in range(nn):
                k0 = 0
                for c, csz in enumerate(_chunk_sizes(nk, n == 0)):
                    # w0c0 alone on sync; everything else (x first) on the
                    # scalar queue — the two queues' DGE completion lags then
                    # overlap instead of retiring serially on one ring
                    weng = nc.sync if (n == 0 and c == 0) else nc.scalar
                    if n == 0:
                        # x K-tiles land just before the W chunk that needs them
                        for k in range(k0, k0 + csz):
                            x_b = xb_pool.tile([KB, nsh], bf16, tag=f"xb{k}")
                            nc.scalar.dma_start(x_b[:, :], xt[k * KB:(k + 1) * KB, :])
                            xb.append(x_b)
                    w_c = wb_pool.tile([KB, csz, NB], bf16, tag=f"wb{n}_{c}")
                    weng.dma_start(w_c[:, :, :], wt4[n, :, k0:k0 + csz, :])
                    strip_chunks[n].append(w_c)
                    for kl in range(csz):
                        wb[n, k0 + kl] = (w_c, kl)
                    k0 += csz
                if n == 0:
                    # bias lands well before the first eviction needs it
                    nc.scalar.dma_start(
                        bias_bc[:, :], bias[None, :].broadcast_to([128, dout])
                    )

            # binarize on the DVE; emitted so strip-n evictions never queue
            # behind later strips' binarize
            def binarize(n):
                for w_c in strip_chunks[n]:
                    nc.vector.tensor_scalar(
                        w_c[:, :, :], w_c[:, :, :], 0.0, 0.5,
                        mybir.AluOpType.is_ge, mybir.AluOpType.subtract,
                    )

            binarize(0)
            if nn > 1:
                binarize(1)

            # GEMM. Strip 0 runs K-outer across nm PSUM banks so the TensorE
            # consumes W chunks as they stream in; later strips (everything
            # resident) run m-outer/K-inner so each PSUM group evicts well
            # before the strip ends — the eviction chain and the next strip's
            # bank-free waits hide entirely behind the matmul stream.
            ev = 0

            def evict(ps_m, m, n):
                nonlocal ev
                ot = out_pool.tile([MB, NB], f32, tag="out")
                nc.vector.scalar_tensor_tensor(
                    ot[:, :], ps_m[:, :], 2.0,
                    bias_bc[:, n * NB:(n + 1) * NB],
                    mybir.AluOpType.mult, mybir.AluOpType.add,
                )
                oeng = nc.sync if ev % 2 == 0 else nc.scalar
                oeng.dma_start(
                    y[m * MB:(m + 1) * MB, n * NB:(n + 1) * NB], ot[:, :]
                )
                ev += 1

            for n in range(nn):
                if n == 0:
                    ps = [
                        psum_pool.tile([MB, NB], f32, tag="ps", name=f"ps0_{m}")
                        for m in range(nm)
                    ]
                    for k in range(nk):
                        w_c, kl = wb[n, k]
                        for m in range(nm):
                            nc.tensor.matmul(
                                ps[m][:, :],
                                xb[k][:, m * MB:(m + 1) * MB],
                                w_c[:, kl, :],
                                start=(k == 0),
                                stop=(k == nk - 1),
                            )
                    for m in range(nm):
                        evict(ps[m], m, n)
                else:
                    for m in range(nm):
                        ps_m = psum_pool.tile(
                            [MB, NB], f32, tag="ps", name=f"ps_{n}_{m}"
                        )
                        for k in range(nk):
                            w_c, kl = wb[n, k]
                            nc.tensor.matmul(
                                ps_m[:, :],
                                xb[k][:, m * MB:(m + 1) * MB],
                                w_c[:, kl, :],
                                start=(k == 0),
                                stop=(k == nk - 1),
                            )
                        evict(ps_m, m, n)
                if n + 2 < nn:
                    binarize(n + 2)
    nc.compile()
    return nc


def _get_nc():
    if "nc" not in _cache:
        _cache["nc"] = build_nc()
    return _cache["nc"]


def run_spmd(nc, in_maps, trace=False):
    from concourse.bass_utils import run_bass_kernel_spmd

    return run_bass_kernel_spmd(
        nc, in_maps, list(range(N_CORES)), trace=trace
    )


def pack_w(weight, din=D_IN, dout=D_OUT):
    """weight [out, in] f32 -> [n_strip, partition, k, col] bf16, contiguous."""
    nk = din // KB
    nn = dout // NB
    a = weight.T.astype(ml_dtypes.bfloat16)           # [in, out]
    a = a.reshape(nk, KB, nn, NB)                     # [k, p, n, j]
    return np.ascontiguousarray(a.transpose(2, 1, 0, 3))


def _in_maps(x, weight, bias):
    x = np.asarray(x, dtype=np.float32)
    weight = np.asarray(weight, dtype=np.float32)
    bias = np.asarray(bias, dtype=np.float32)
    wt4 = pack_w(weight)
    maps = []
    for i in range(N_CORES):
        xs = np.ascontiguousarray(
            x[i * N_SH:(i + 1) * N_SH].T.astype(ml_dtypes.bfloat16)
        )
        maps.append({"xt": xs, "wt4": wt4, "bias": bias})
    return maps


def kernel(x, weight, bias):
    nc = _get_nc()
    res = run_spmd(nc, _in_maps(x, weight, bias))
    y = np.concatenate([res.results[i]["y"] for i in range(N_CORES)], axis=0)
    return np.ascontiguousarray(y.astype(np.float32))


# revision 27
# speedup vs baseline: 1.0050x; 1.0050x over previous
"""BinaryLinear Trainium2 kernel: y = x @ sign(W).T + bias.

Full shapes: x [8192, 2048] f32, W [2048, 2048] f32, bias [2048] f32.
Strategy: data-parallel over 8 NeuronCores — shard x rows (1024/core),
replicate W and bias, no collectives. Host only shards / lays out /
down-casts to the kernel's bf16 compute precision (sign is preserved
exactly by the bf16 cast); all math (sign, matmul, bias add) runs on
device.

Numerics: W is binarized on-device to {-0.5, +0.5} in bf16 via
(w >= 0) - 0.5 (one DVE op, in place); the missing factor 2 is folded
into the fp32 PSUM eviction (out = 2*psum + bias, one DVE op). Both
factors are powers of two, so the result equals x*sign(W) exactly up to
the single bf16 rounding of x. Accumulation is fp32 in PSUM (K=2048).

Schedule: W streams in 512-out-col strips, host-packed as
[strip, partition, k, col] so every DMA line is >=2KB contiguous. Each
strip is computed K-outer across 8 PSUM banks (one per 128-row x
block), so the TensorE consumes chunks as they arrive and never waits
on the full W. Only the first W chunk + two x K-tiles ride the sync DMA
queue (kept shallow — DGE completions retire in order, so a deep ring
delays the critical first tiles); the bulk streams on the scalar
engine's queue in consumption order. Binarize is emitted so strip-n
evictions never queue behind later strips' binarize on the DVE. Warmup
matmuls on a scratch tile lift the PE clock gate before real data
lands. Output DMAs alternate between the sync and scalar HWDGE queues
(SWDGE drain at kernel end is slow).
"""

import numpy as np
import ml_dtypes

N_CORES = 8
N_ROWS = 8192
D_IN = 2048
D_OUT = 2048
N_SH = N_ROWS // N_CORES

KB = 128            # contraction block (SBUF partitions)
MB = 128            # x-row block (stationary free dim -> out partitions)
NB = 512            # out-col block (moving free dim, one PSUM bank)

_cache = {}


def _chunk_sizes(nk, first_strip):
    # strip 0 uses small leading chunks so the first matmul starts early
    sizes = []
    k = 0
    while k < nk:
        if first_strip:
            csz = 1 if len(sizes) < 2 else 2
        else:
            csz = 8
        s = min(csz, nk - k)
        sizes.append(s)
        k += s
    return sizes


def build_nc(nsh=N_SH, din=D_IN, dout=D_OUT, warmup_mms=9):
    import concourse.bass as bass
    import concourse.bacc as bacc
    import concourse.tile as tile
    from concourse import mybir

    f32 = mybir.dt.float32
    bf16 = mybir.dt.bfloat16

    nk = din // KB
    nm = nsh // MB
    nn = dout // NB
    assert nm <= 8, "one PSUM bank per x-row block"

    nc = bacc.Bacc("TRN2", debug=False)
    xt = nc.dram_tensor("xt", [din, nsh], bf16, kind="ExternalInput").ap()
    wt4 = nc.dram_tensor("wt4", [nn, KB, nk, NB], bf16, kind="ExternalInput").ap()
    bias = nc.dram_tensor("bias", [dout], f32, kind="ExternalInput").ap()
    y = nc.dram_tensor("y", [nsh, dout], f32, kind="ExternalOutput").ap()

    with tile.TileContext(nc) as tc:
        with (
            tc.tile_pool(name="wb", bufs=1) as wb_pool,
            tc.tile_pool(name="xb", bufs=1) as xb_pool,
            tc.tile_pool(name="biasp", bufs=1) as bias_pool,
            tc.tile_pool(name="out", bufs=8) as out_pool,
            tc.tile_pool(name="psum", bufs=8, space=bass.MemorySpace.PSUM) as psum_pool,
        ):
            # PE clock-gate warmup on a zeroed scratch tile
            if warmup_mms:
                dummy = bias_pool.tile([128, NB], bf16, tag="dummy")
                nc.vector.memset(dummy[:, :], 0.0)
                wps = psum_pool.tile([128, NB], f32, tag="ps", name="ps_warm")
                for _ in range(warmup_mms):
                    nc.tensor.matmul(
                        wps[:, :], dummy[:, 0:MB], dummy[:, :],
                        start=True, stop=True,
                    )

            # Input DMAs in exact consumption order. Only the first W chunk
            # and its two x K-tiles go on the sync queue (kept shallow so
            # their completion semaphores retire fast); the rest streams on
            # the scalar engine's queue, self-pacing at full bandwidth.
            bias_bc = bias_pool.tile([128, dout], f32, tag="biasbc")
            xb = []
            wb = {}          # (n, k) -> (chunk tile, local k index)
            strip_chunks = [[] for _ in range(nn)]
            for n in range(nn):
                k0 = 0
                for c, csz in enumerate(_chunk_sizes(nk, n == 0)):
                    # w0c0 alone on sync; everything else (x first) on the
                    # scalar queue — the two queues' DGE completion lags then
                    # overlap instead of retiring serially on one ring
                    weng = nc.sync if (n == 0 and c == 0) else nc.scalar
                    if n == 0:
                        # x K-tiles land just before the W chunk that needs them
                        for k in range(k0, k0 + csz):
                            x_b = xb_pool.tile([KB, nsh], bf16, tag=f"xb{k}")
                            nc.scalar.dma_start(x_b[:, :], xt[k * KB:(k + 1) * KB, :])
                            xb.append(x_b)
                    w_c = wb_pool.tile([KB, csz, NB], bf16, tag=f"wb{n}_{c}")
                    weng.dma_start(w_c[:, :, :], wt4[n, :, k0:k0 + csz, :])
                    strip_chunks[n].append(w_c)
                    for kl in range(csz):
                        wb[n, k0 + kl] = (w_c, kl)
                    k0 += csz
                if n == 0:
                    # bias lands well before the first eviction needs it
                    nc.scalar.dma_start(
                        bias_bc[:, :], bias[None, :].broadcast_to([128, dout])
                    )

            # binarize on the DVE; emitted so strip-n evictions never queue
            # behind later strips' binarize
            def binarize(n):
                for w_c in strip_chunks[n]:
                    nc.vector.tensor_scalar(
                        w_c[:, :, :], w_c[:, :, :], 0.0, 0.5,
                        mybir.AluOpType.is_ge, mybir.AluOpType.subtract,
                    )

            binarize(0)
            if nn > 1:
                binarize(1)

            # GEMM. Strip 0 runs K-outer across nm PSUM banks so the TensorE
            # consumes W chunks as they stream in; later strips (everything
            # resident) run m-outer/K-inner so each PSUM group evicts well
            # before the strip ends — the eviction chain and the next strip's
            # bank-free waits hide entirely behind the matmul stream.
            ev = 0

            def evict(ps_m, m, n):
                nonlocal ev
                ot = out_pool.tile([MB, NB], f32, tag="out")
                nc.vector.scalar_tensor_tensor(
                    ot[:, :], ps_m[:, :], 2.0,
                    bias_bc[:, n * NB:(n + 1) * NB],
                    mybir.AluOpType.mult, mybir.AluOpType.add,
                )
                oeng = nc.sync if ev % 2 == 0 else nc.scalar
                oeng.dma_start(
                    y[m * MB:(m + 1) * MB, n * NB:(n + 1) * NB], ot[:, :]
                )
                ev += 1

            for n in range(nn):
                if n == 0:
                    ps = [
                        psum_pool.tile([MB, NB], f32, tag="ps", name=f"ps0_{m}")
                        for m in range(nm)
                    ]
                    for k in range(nk):
                        w_c, kl = wb[n, k]
                        for m in range(nm):
                            nc.tensor.matmul(
                                ps[m][:, :],
                                xb[k][:, m * MB:(m + 1) * MB],
                                w_c[:, kl, :],
                                start=(k == 0),
                                stop=(k == nk - 1),
                            )
                    for m in range(nm):
                        evict(ps[m], m, n)
                else:
                    for m in range(nm):
                        ps_m = psum_pool.tile(
                            [MB, NB], f32, tag="ps", name=f"ps_{n}_{m}"
                        )
                        for k in range(nk):
                            w_c, kl = wb[n, k]
                            nc.tensor.matmul(
                                ps_m[:, :],
                                xb[k][:, m * MB:(m + 1) * MB],
                                w_c[:, kl, :],
                                start=(k == 0),
                                stop=(k == nk - 1),
                            )
                        evict(ps_m, m, n)
                if n + 2 < nn:
                    binarize(n + 2)
    nc.compile()
    return nc


def _get_nc():
    if "nc" not in _cache:
        _cache["nc"] = build_nc()
    return _cache["nc"]


def run_spmd(nc, in_maps, trace=False):
    from concourse.bass_utils import run_bass_kernel_spmd

    return run_bass_kernel_spmd(
        nc, in_maps, list(range(N_CORES)), trace=trace
    )


def pack_w(weight, din=D_IN, dout=D_OUT):
    """weight [out, in] f32 -> [n_strip, partition, k, col] bf16, contiguous."""
    nk = din // KB
    nn = dout // NB
    a = weight.T.astype(ml_dtypes.bfloat16)           # [in, out]
    a = a.reshape(nk, KB, nn, NB)                     # [k, p, n, j]
    return np.ascontiguousarray(a.transpose(2, 1, 0, 3))


def _in_maps(x, weight, bias):
    x = np.asarray(x, dtype=np.float32)
    weight = np.asarray(weight, dtype=np.float32)
    bias = np.asarray(bias, dtype=np.float32)
    wt4 = pack_w(weight)
    maps = []
    for i in range(N_CORES):
        xs = np.ascontiguousarray(
            x[i * N_SH:(i + 1) * N_SH].T.astype(ml_dtypes.bfloat16)
        )
        maps.append({"xt": xs, "wt4": wt4, "bias": bias})
    return maps


def kernel(x, weight, bias):
    nc = _get_nc()
    res = run_spmd(nc, _in_maps(x, weight, bias))
    y = np.concatenate([res.results[i]["y"] for i in range(N_CORES)], axis=0)
    return np.ascontiguousarray(y.astype(np.float32))
